# revision 30
# baseline (speedup 1.0000x reference)
"""DeChunk layer kernel for Trainium2 (8 NeuronCores, Bass/Tile).

Reference semantics (per batch row b):
    p = clip(boundary_prob[b,:,1], EPS, 1-EPS)
    p_chunked[m] = p at the (m+1)-th boundary position (argsort compaction)
    expanded[0] = x[0]; expanded[m] = pc[m]*x[m] + (1-pc[m])*expanded[m-1]
    out[l] = expanded[clip(cumsum(mask)[l]-1, 0, M-1)]

Sharding: 8 cores = (batch b = core//2) x (D-half = core%2); no collectives.

V6: token-domain blocked scan with:
1. Window gather: within a 128-token block the boundary tokens consume x
   rows r0_g, r0_g+1, ... in order, so one indirect DMA per block fetches
   just that contiguous window (OOB-skip offsets for unused slots): gather
   traffic <=M rows (2 MiB bf16) per core instead of L rows (16 MiB f32).
   A per-block one-hot compose matmul N'[w,t] = sum_s ST[s,w]*lh[s,t]
   (ST[s,w] = pp[s]*[lidx[s]==w]) maps window rows to scan coefficients.
2. bf16 operands on the PE (f32 PSUM accumulate) and bf16 output rows
   (host upcasts), halving output write traffic.
3. Cross-block carry folded into the compose matmul: window row 127 is
   loaded with the solved carry row and N'[127,:] += g2[t] via a rank-1
   matmul into the same PSUM group (valid while every block has <=127
   boundaries; asserted host-side). Block-local tails S_g for the parallel
   carry solve (Lb system) accumulate batched into an [8, DSH] PSUM.
4. c[t] broadcast across partitions via the gpsimd partition_broadcast
   custom op on a DRAM-bounced row (contiguous per-partition stores only:
   scattered 4-byte-descriptor DMAs serialize on one DMA engine).
"""

import numpy as np

B, L, M, D = 4, 4096, 1024, 2048
NCORES = 8
DSH = D // 2          # per-core D slice
EPS = 1e-4
NB = L // 128         # 32 token blocks of 128 tokens
NEG = 88.0            # exp(-88) ~= 0 for triangular masking
CLAMP = 8.75651076e-27  # exp(-60): floor for a' so ln stays finite
LAG = 7               # blocks between gather/compose and final matmul
XW_BUFS = 10          # window-tile pool depth; first rotation uses clipped
                      # (always-valid) offsets so later OOB-skipped slots
                      # only alias stale-but-finite gathered data
FIRSTROT = XW_BUFS
MM1024 = False        # 1024-wide matmuls fail the s3d3_mm_num_elements ISA
                      # check (PSUM-bank limit) — keep 2x512

_PROGRAM = None


def build_program_v6():
    import concourse.bass as bass
    import concourse.bacc as bacc
    import concourse.mybir as mybir
    from concourse.tile import TileContext
    from concourse.masks import make_identity, make_upper_triangular

    f32 = mybir.dt.float32
    bf16 = mybir.dt.bfloat16
    i32 = mybir.dt.int32
    u8 = mybir.dt.uint8
    Alu = mybir.AluOpType
    Act = mybir.ActivationFunctionType

    nc = bacc.Bacc("TRN2", target_bir_lowering=False)
    x_d = nc.declare_dram_parameter("x", [M, DSH], bf16, isOutput=False)
    prob_d = nc.declare_dram_parameter("prob", [L, 2], f32, isOutput=False)
    mask_d = nc.declare_dram_parameter("mask", [L], u8, isOutput=False)
    out_d = nc.declare_dram_parameter("out", [L, DSH], bf16, isOutput=True)

    with TileContext(nc) as tc:
        with (
            tc.tile_pool(name="const", bufs=1) as constp,
            tc.tile_pool(name="small", bufs=1) as small,
            tc.tile_pool(name="xw", bufs=XW_BUFS) as xwp,
            tc.tile_pool(name="npr", bufs=10) as nprp,
            tc.tile_pool(name="blk", bufs=3) as blkp,
            tc.tile_pool(name="msk", bufs=2) as mskp,
            tc.tile_pool(name="eo", bufs=4) as eop,
            tc.tile_pool(name="ps_main", bufs=2, space="PSUM") as psm,
            tc.tile_pool(name="ps_sbatch", bufs=1, space="PSUM") as pssb,
            tc.tile_pool(name="ps_x", bufs=2, space="PSUM") as psb,
            tc.tile_pool(name="dram", bufs=1, space="DRAM") as dramp,
        ):
            # ---- constants ----
            ident = constp.tile([128, 128], f32, tag="ident")
            make_identity(nc, ident[:])
            u_incl = constp.tile([128, 128], f32, tag="u_incl")   # [q <= p]
            make_upper_triangular(nc, u_incl[:], val=1.0, diag=True)
            ones_row = constp.tile([1, 128], f32, tag="ones_row")
            nc.gpsimd.memset(ones_row[:], 1.0)
            negb = constp.tile([128, 1], f32, tag="negb")
            nc.gpsimd.memset(negb[:], -NEG)
            zcol = constp.tile([128, 1], f32, tag="zcol")
            nc.gpsimd.memset(zcol[:], 0.0)
            nc.const_aps.aps[(f32, 0.0)] = zcol[:]
            nc.const_aps.aps[(f32, -NEG)] = negb[:]
            # e127: one-hot at position 127 (row form, f32 + bf16)
            e127f = constp.tile([1, 128], f32, tag="e127f")
            nc.gpsimd.memset(e127f[:], 0.0)
            nc.gpsimd.memset(e127f[0:1, 127:128], 1.0)
            e127b = constp.tile([1, 128], bf16, tag="e127b")
            nc.vector.tensor_copy(out=e127b[:], in_=e127f[:])
            # iota columns / broadcast rows
            iota_col = constp.tile([128, 1], f32, tag="iota_col")
            nc.gpsimd.iota(iota_col[:], pattern=[[1, 1]], base=0,
                           channel_multiplier=1,
                           allow_small_or_imprecise_dtypes=True)
            iota_bc = constp.tile([128, 128], f32, tag="iota_bc")
            nc.gpsimd.iota(iota_bc[:], pattern=[[1, 128]], base=0,
                           channel_multiplier=0,
                           allow_small_or_imprecise_dtypes=True)
            iota8_bc = constp.tile([128, 8], f32, tag="iota8_bc")
            nc.gpsimd.iota(iota8_bc[:], pattern=[[1, 8]], base=0,
                           channel_multiplier=0,
                           allow_small_or_imprecise_dtypes=True)

            # ---- DRAM scratch for the two coefficient-row bounces ----
            c2d = dramp.tile([L], f32, tag="c2d")
            g2d = dramp.tile([L], bf16, tag="g2d")

            # ---- loads: [32 blocks = partitions, 128 tokens = cols] ----
            A_prob = small.tile([NB, 256], f32, tag="A_prob")
            nc.sync.dma_start(
                out=A_prob[:],
                in_=prob_d[:].rearrange("(g s) c -> g (s c)", g=NB),
            )
            A_mask = small.tile([NB, 128], u8, tag="A_mask")
            nc.sync.dma_start(
                out=A_mask[:], in_=mask_d[:].rearrange("(g s) -> g s", g=NB)
            )
            pv = A_prob[:].rearrange("g (s c) -> g s c", c=2)
            Ap2 = small.tile([NB, 128], f32, tag="Ap2")
            nc.vector.tensor_scalar(
                out=Ap2[:], in0=pv[:, :, 1], scalar1=float(EPS),
                scalar2=float(1.0 - EPS), op0=Alu.max, op1=Alu.min,
            )
            Am2 = small.tile([NB, 128], f32, tag="Am2")
            nc.vector.tensor_copy(out=Am2[:], in_=A_mask[:])

            # transpose to p-minor [128 tokens, NB blocks]
            tm_ps = psb.tile([128, NB], f32, space="PSUM", tag="x")
            nc.tensor.transpose(out=tm_ps[:], in_=Am2[:], identity=ident[:NB, :NB])
            m_pm = small.tile([128, NB], f32, tag="m_pm")
            nc.vector.tensor_copy(out=m_pm[:], in_=tm_ps[:])
            tp_ps = psb.tile([128, NB], f32, space="PSUM", tag="x")
            nc.tensor.transpose(out=tp_ps[:], in_=Ap2[:], identity=ident[:NB, :NB])
            p_pm = small.tile([128, NB], f32, tag="p_pm")
            nc.vector.tensor_copy(out=p_pm[:], in_=tp_ps[:])

            # ---- block sums and global inclusive counts ----
            ocol = constp.tile([128, 1], f32, tag="ocol")
            nc.gpsimd.memset(ocol[:], 1.0)
            s_ps = psb.tile([1, NB], f32, space="PSUM", tag="x")
            nc.tensor.matmul(out=s_ps[:], lhsT=ocol[:], rhs=m_pm[:],
                             start=True, stop=True)
            s_sb = small.tile([1, NB], f32, tag="s_sb")
            nc.vector.tensor_copy(out=s_sb[:], in_=s_ps[:])
            sinc = small.tile([1, NB], f32, tag="sinc")
            nc.vector.tensor_tensor_scan(
                out=sinc[:], data0=s_sb[:], data1=s_sb[:],
                initial=0.0, op0=Alu.add, op1=Alu.max,
            )
            sex = small.tile([1, NB], f32, tag="sex")
            nc.vector.tensor_tensor(out=sex[:], in0=sinc[:], in1=s_sb[:],
                                    op=Alu.subtract)
            cnt_ps = psb.tile([128, NB], f32, space="PSUM", tag="x")
            nc.tensor.matmul(out=cnt_ps[:], lhsT=u_incl[:], rhs=m_pm[:],
                             start=True, stop=False)
            nc.tensor.matmul(out=cnt_ps[:], lhsT=ones_row[:], rhs=sex[:],
                             start=False, stop=True)
            cnt = small.tile([128, NB], f32, tag="cnt")
            nc.vector.tensor_copy(out=cnt[:], in_=cnt_ps[:])

            # ---- CRITICAL-PATH HEAD: pp -> a' -> ln -> c2 -> bounce/bcast --
            sel = small.tile([128, NB], f32, tag="sel")
            nc.vector.tensor_scalar(
                out=sel[:], in0=cnt[:], scalar1=2.0, scalar2=None, op0=Alu.is_ge,
            )
            tM = small.tile([128, NB], f32, tag="tM")
            nc.vector.tensor_scalar(
                out=tM[:], in0=cnt[:], scalar1=float(M), scalar2=None,
                op0=Alu.is_le,
            )
            nc.vector.tensor_tensor(out=sel[:], in0=sel[:], in1=tM[:], op=Alu.mult)
            nc.vector.tensor_tensor(out=sel[:], in0=sel[:], in1=m_pm[:],
                                    op=Alu.mult)
            pp = small.tile([128, NB], f32, tag="pp")
            nc.vector.tensor_tensor(out=pp[:], in0=p_pm[:], in1=sel[:],
                                    op=Alu.mult)
            teq = small.tile([128, NB], f32, tag="teq")
            nc.vector.tensor_scalar(
                out=teq[:], in0=cnt[:], scalar1=1.0, scalar2=None,
                op0=Alu.is_equal,
            )
            nc.vector.tensor_tensor(out=teq[:], in0=teq[:], in1=m_pm[:],
                                    op=Alu.mult)
            nc.vector.tensor_tensor(out=pp[:], in0=pp[:], in1=teq[:], op=Alu.add)
            ap_ = small.tile([128, NB], f32, tag="ap_")
            nc.vector.tensor_scalar(
                out=ap_[:], in0=pp[:], scalar1=-1.0, scalar2=1.0,
                op0=Alu.mult, op1=Alu.add,
            )
            nc.vector.tensor_scalar(
                out=ap_[:], in0=ap_[:], scalar1=CLAMP, scalar2=None, op0=Alu.max,
            )
            la = small.tile([128, NB], f32, tag="la")
            nc.scalar.activation(out=la[:], in_=ap_[:], func=Act.Ln)

            c2_ps = psb.tile([128, NB], f32, space="PSUM", tag="x")
            nc.tensor.matmul(out=c2_ps[:], lhsT=u_incl[:], rhs=la[:],
                             start=True, stop=True)
            c2_pm = small.tile([128, NB], f32, tag="c2_pm")
            nc.vector.tensor_copy(out=c2_pm[:], in_=c2_ps[:])
            negc2 = small.tile([128, NB], f32, tag="negc2")
            nc.vector.tensor_scalar_mul(out=negc2[:], in0=c2_pm[:], scalar1=-1.0)

            c2T_ps = psb.tile([NB, 128], f32, space="PSUM", tag="x")
            nc.tensor.transpose(out=c2T_ps[:], in_=c2_pm[:],
                                identity=ident[:128, :128])
            c2T = small.tile([NB, 128], f32, tag="c2T")
            nc.vector.tensor_copy(out=c2T[:], in_=c2T_ps[:])
            g2T = small.tile([NB, 128], bf16, tag="g2T")
            nc.scalar.activation(out=g2T[:], in_=c2T[:], func=Act.Exp)
            nc.sync.dma_start(
                out=c2d[:].rearrange("(g s) -> g s", g=NB), in_=c2T[:]
            )
            nc.scalar.dma_start(
                out=g2d[:].rearrange("(g s) -> g s", g=NB), in_=g2T[:]
            )
            c2row = small.tile([1, L], f32, tag="c2row")
            nc.sync.dma_start(
                out=c2row[:], in_=c2d[:].rearrange("(o n) -> o n", o=1)
            )
            g2row = small.tile([1, L], bf16, tag="g2row")
            nc.scalar.dma_start(
                out=g2row[:], in_=g2d[:].rearrange("(o n) -> o n", o=1)
            )
            cbc = small.tile([128, L], f32, tag="cbc")
            for q in range(4):
                nc.gpsimd.partition_broadcast(
                    cbc[:, q * 1024:(q + 1) * 1024],
                    c2row[0:1, q * 1024:(q + 1) * 1024],
                )

            # ---- gather offsets ----
            cm1 = small.tile([128, NB], f32, tag="cm1")
            nc.vector.tensor_scalar_add(out=cm1[:], in0=cnt[:], scalar1=-1.0)
            rbc_ps = psb.tile([128, NB], f32, space="PSUM", tag="x")
            nc.tensor.matmul(out=rbc_ps[:], lhsT=ones_row[:], rhs=sex[:],
                             start=True, stop=True)
            rbc = small.tile([128, NB], f32, tag="rbc")
            nc.vector.tensor_copy(out=rbc[:], in_=rbc_ps[:])
            sbc_ps = psb.tile([128, NB], f32, space="PSUM", tag="x")
            nc.tensor.matmul(out=sbc_ps[:], lhsT=ones_row[:], rhs=sinc[:],
                             start=True, stop=True)
            sbc = small.tile([128, NB], f32, tag="sbc")
            nc.vector.tensor_copy(out=sbc[:], in_=sbc_ps[:])
            lidx = small.tile([128, NB], f32, tag="lidx")
            nc.vector.tensor_tensor(out=lidx[:], in0=cm1[:], in1=rbc[:],
                                    op=Alu.subtract)
            off0 = small.tile([128, NB], f32, tag="off0")
            nc.vector.tensor_scalar_add(out=off0[:], in0=rbc[:],
                                        scalar1=iota_col[:])
            vnb = small.tile([128, NB], f32, tag="vnb")
            nc.vector.tensor_tensor(out=vnb[:], in0=off0[:], in1=sbc[:],
                                    op=Alu.is_lt)
            vM = small.tile([128, NB], f32, tag="vM")
            nc.vector.tensor_scalar(
                out=vM[:], in0=off0[:], scalar1=float(M - 1), scalar2=None,
                op0=Alu.is_le,
            )
            nc.vector.tensor_tensor(out=vnb[:], in0=vnb[:], in1=vM[:],
                                    op=Alu.mult)
            offm = small.tile([128, NB], f32, tag="offm")
            nc.vector.tensor_scalar_add(out=offm[:], in0=off0[:],
                                        scalar1=float(-M))
            nc.vector.tensor_tensor(out=offm[:], in0=offm[:], in1=vnb[:],
                                    op=Alu.mult)
            nc.vector.tensor_scalar_add(out=offm[:], in0=offm[:],
                                        scalar1=float(M))
            offc = small.tile([128, NB], f32, tag="offc")
            nc.vector.tensor_scalar(
                out=offc[:], in0=off0[:], scalar1=float(M - 1), scalar2=0.0,
                op0=Alu.min, op1=Alu.max,
            )
            ci_i = small.tile([128, NB], i32, tag="ci_i")
            nc.vector.tensor_copy(out=ci_i[:, 0:FIRSTROT], in_=offc[:, 0:FIRSTROT])
            nc.vector.tensor_copy(out=ci_i[:, FIRSTROT:NB], in_=offm[:, FIRSTROT:NB])

            # ---- Lb (carry propagation, NB x NB incl virtual x[0] row) ----
            e127c_ps = psb.tile([128, 1], f32, space="PSUM", tag="x")
            nc.tensor.transpose(out=e127c_ps[:], in_=e127f[:],
                                identity=ident[:1, :1])
            e127c = small.tile([128, 1], f32, tag="e127c")
            nc.vector.tensor_copy(out=e127c[:], in_=e127c_ps[:])
            lgB_ps = psb.tile([1, NB], f32, space="PSUM", tag="x")
            nc.tensor.matmul(out=lgB_ps[:], lhsT=e127c[:], rhs=c2_pm[:],
                             start=True, stop=True)
            lgB = small.tile([1, NB], f32, tag="lgB")
            nc.vector.tensor_copy(out=lgB[:], in_=lgB_ps[:])
            hb = small.tile([1, NB], f32, tag="hb")
            nc.vector.tensor_tensor_scan(
                out=hb[:], data0=lgB[:], data1=lgB[:],
                initial=0.0, op0=Alu.add, op1=Alu.min,
            )
            hbx = small.tile([1, NB], f32, tag="hbx")
            nc.vector.tensor_copy(out=hbx[:, 1:NB], in_=hb[:, 0:NB - 1])
            nc.vector.memset(hbx[:, 0:1], 0.0)
            nhx_ps = psb.tile([NB, 1], f32, space="PSUM", tag="x")
            nc.tensor.transpose(out=nhx_ps[:], in_=hbx[:], identity=ident[:1, :1])
            neghbx = small.tile([NB, 1], f32, tag="neghbx")
            nc.vector.tensor_scalar_mul(out=neghbx[:], in0=nhx_ps[:], scalar1=-1.0)
            lb_ps = psb.tile([NB, NB], f32, space="PSUM", tag="x")
            nc.tensor.matmul(out=lb_ps[:], lhsT=ones_row[0:1, 0:NB], rhs=hbx[:],
                             start=True, stop=True)
            lbs = small.tile([NB, NB], f32, tag="lbs")
            nc.vector.tensor_scalar(
                out=lbs[:], in0=lb_ps[:], scalar1=neghbx[:], scalar2=NEG,
                op0=Alu.add, op1=Alu.add,
            )
            nc.vector.tensor_tensor(out=lbs[:], in0=lbs[:],
                                    in1=u_incl[:NB, :NB], op=Alu.mult)
            LbT = small.tile([NB, NB], bf16, tag="LbT")
            nc.scalar.activation(out=LbT[:], in_=lbs[:], func=Act.Exp, bias=-NEG)

            # ---- S rows (block tails; row 0 = virtual x[0]) ----
            S_sb = small.tile([NB, DSH], bf16, tag="S_sb")
            nc.sync.dma_start(out=S_sb[0:1, :], in_=x_d[0:1, :])
            C_sbs = []
            xws = {}
            nprs = {}
            sb_ps_ref = [None]

            def final_block(ff, qi):
                k = ff // 8
                xw = xws.pop(ff)
                # carry row -> window slot 127 (never gather-written when
                # nb_g <= 127, which the host asserts)
                eng_cl = nc.sync if qi % 2 == 0 else nc.scalar
                eng_cl.dma_start(
                    out=xw[127:128, :],
                    in_=C_sbs[k][ff % 8:ff % 8 + 1, :],
                )
                npr = nprs.pop(ff)
                ps = psm.tile([128, DSH], f32, space="PSUM", tag="main")
                if MM1024:
                    nc.tensor.matmul(out=ps[:], lhsT=npr[:], rhs=xw[:],
                                     start=True, stop=True)
                else:
                    for h in range(DSH // 512):
                        cs = slice(h * 512, (h + 1) * 512)
                        nc.tensor.matmul(
                            out=ps[:, cs], lhsT=npr[:], rhs=xw[:, cs],
                            start=True, stop=True,
                        )
                eo = eop.tile([128, DSH], bf16, tag="eo")
                if qi % 2:
                    nc.scalar.activation(out=eo[:], in_=ps[:], func=Act.Copy)
                else:
                    nc.vector.tensor_copy(out=eo[:], in_=ps[:])
                eng_out = nc.sync if qi % 2 else nc.scalar
                eng_out.dma_start(out=out_d[ff * 128:(ff + 1) * 128, :], in_=eo[:])

            for g in range(NB):
                # -- window gather --
                xw = xwp.tile([128, DSH], bf16, tag="xw")
                nc.gpsimd.indirect_dma_start(
                    out=xw[:], out_offset=None, in_=x_d[:, :],
                    in_offset=bass.IndirectOffsetOnAxis(
                        ap=ci_i[:, g:g + 1], axis=0
                    ),
                    bounds_check=M - 1,
                    oob_is_err=False,
                )
                xws[g] = xw

                # -- lh[s, t] = exp(c[t] - c[s]) on the triangle --
                dsb = blkp.tile([128, 128], f32, tag="dsb")
                nc.vector.tensor_scalar(
                    out=dsb[:], in0=cbc[:, g * 128:(g + 1) * 128],
                    scalar1=negc2[:, g:g + 1],
                    scalar2=NEG, op0=Alu.add, op1=Alu.add,
                )
                nc.vector.tensor_tensor(out=dsb[:], in0=dsb[:], in1=u_incl[:],
                                        op=Alu.mult)
                lh = blkp.tile([128, 128], bf16, tag="lh")
                nc.scalar.activation(out=lh[:], in_=dsb[:], func=Act.Exp,
                                     bias=-NEG)

                # -- ST one-hot: st[s, w] = [lidx[s] == w] * pp[s] (fused) --
                st = blkp.tile([128, 128], bf16, tag="st")
                nc.vector.tensor_scalar(
                    out=st[:], in0=iota_bc[:], scalar1=lidx[:, g:g + 1],
                    scalar2=pp[:, g:g + 1], op0=Alu.is_equal, op1=Alu.mult,
                )

                # -- compose N' = ST.T @ lh + e127 (x) g2row --
                n_ps = psb.tile([128, 128], f32, space="PSUM", tag="x")
                nc.tensor.matmul(out=n_ps[:], lhsT=st[:], rhs=lh[:],
                                 start=True, stop=False)
                nc.tensor.matmul(
                    out=n_ps[:], lhsT=e127b[:],
                    rhs=g2row[0:1, g * 128:(g + 1) * 128],
                    start=False, stop=True,
                )
                npr = nprp.tile([128, 128], bf16, tag="npr")
                nc.scalar.activation(out=npr[:], in_=n_ps[:], func=Act.Copy)
                nprs[g] = npr
                npcol = mskp.tile([128, 1], f32, tag="npcol")
                nc.vector.tensor_copy(out=npcol[:], in_=n_ps[:, 127:128])

                # -- batched block tails S (rows (g+1)%8 of an [8, DSH] psum)
                if g <= NB - 2:
                    r = (g + 1) % 8
                    if r == 0 or g == 0:
                        sb_ps_ref[0] = pssb.tile([8, DSH], f32, space="PSUM",
                                                 tag="sbatch",
                                                 name=f"sb_ps{(g + 1) // 8}")
                    sb_ps = sb_ps_ref[0]
                    msk8 = mskp.tile([128, 8], bf16, tag="msk8")
                    nc.vector.tensor_scalar(
                        out=msk8[:], in0=iota8_bc[:], scalar1=float(r),
                        scalar2=npcol[:], op0=Alu.is_equal,
                        op1=Alu.mult,
                    )
                    first = (r == 0 or g == 0)
                    last = (r == 7)
                    if MM1024:
                        nc.tensor.matmul(
                            out=sb_ps[:], lhsT=msk8[0:127, :],
                            rhs=xw[0:127, :], start=first, stop=last,
                        )
                    else:
                        for h in range(DSH // 512):
                            cs = slice(h * 512, (h + 1) * 512)
                            nc.tensor.matmul(
                                out=sb_ps[:, cs], lhsT=msk8[0:127, :],
                                rhs=xw[0:127, cs], start=first, stop=last,
                            )
                    if last:
                        k = (g - 6) // 8
                        sch = blkp.tile([8, DSH], bf16, tag="sch")
                        nc.vector.tensor_copy(out=sch[:], in_=sb_ps[:])
                        if k == 0:
                            # row 0 is the virtual x[0] row — keep it
                            nc.sync.dma_start(out=S_sb[1:8, :], in_=sch[1:8, :])
                        else:
                            nc.sync.dma_start(
                                out=S_sb[8 * k:8 * k + 8, :], in_=sch[:]
                            )

                # -- carry chunk solve every 8 blocks --
                k = (g - 6) // 8
                if g >= 6 and (g - 6) % 8 == 0 and k <= 3:
                    kk = k * 8 + 8
                    C_sb = small.tile([8, DSH], bf16, tag=f"C_sb{k}")
                    for h in range(DSH // 256):
                        cs = slice(h * 256, (h + 1) * 256)
                        C_ps = psb.tile([8, 256], f32, space="PSUM",
                                        tag="x")
                        nc.tensor.matmul(
                            out=C_ps[:], lhsT=LbT[0:kk, k * 8:k * 8 + 8],
                            rhs=S_sb[0:kk, cs], start=True, stop=True,
                        )
                        nc.vector.tensor_copy(out=C_sb[:, cs], in_=C_ps[:])
                    C_sbs.append(C_sb)

                if g >= LAG:
                    final_block(g - LAG, g)
            for gg in range(NB - LAG, NB):
                final_block(gg, gg)

    nc.compile()
    return nc


def _get_program():
    global _PROGRAM
    if _PROGRAM is None:
        _PROGRAM = build_program_v6()
    return _PROGRAM


def make_in_maps(chunked_states, boundary_prob, boundary_mask):
    import ml_dtypes

    mask_u8 = np.ascontiguousarray(boundary_mask).astype(np.uint8)
    # carry-fold safety: no 128-token block may have 128 boundaries
    nbs = mask_u8.reshape(B, NB, 128).sum(axis=2)
    assert nbs.max() <= 127, "carry-fold invalid: a block has 128 boundaries"
    in_maps = []
    for c in range(NCORES):
        b, h = c // 2, c % 2
        in_maps.append({
            "x": np.ascontiguousarray(
                chunked_states[b, :, h * DSH:(h + 1) * DSH]
            ).astype(ml_dtypes.bfloat16),
            "prob": np.ascontiguousarray(boundary_prob[b], dtype=np.float32),
            "mask": mask_u8[b],
        })
    return in_maps


def assemble(results):
    out = np.empty((B, L, D), np.float32)
    for c in range(NCORES):
        b, h = c // 2, c % 2
        out[b, :, h * DSH:(h + 1) * DSH] = results[c]["out"].astype(np.float32)
    return out


def kernel(chunked_states, boundary_prob, boundary_mask):
    from concourse.bass_utils import run_bass_kernel_spmd

    nc = _get_program()
    in_maps = make_in_maps(chunked_states, boundary_prob, boundary_mask)
    res = run_bass_kernel_spmd(nc, in_maps, list(range(NCORES)))
    return assemble(res.results)
